# revision 19
# baseline (speedup 1.0000x reference)
"""BEM (boundary evaluation module) Trainium2 kernel, v3: t-sharded SPMD.

Each of the 8 cores owns a 32-wide slice of the T=256 axis.  Stage 1
(conv1+GN1, ~2% of the FLOPs) is computed redundantly on every core; the
expensive sampling GEMM and Conv3d reduction (stages 2-3) and the whole
tail run only on each core's 34-wide haloed window (+-1 t for the heads'
3x3 convs).  GroupNorm statistics are global over (T, W), so the three GN
points after stage 3 exchange per-core partial moments with tiny (<=1KB)
AllReduces (~1-2us each measured).  Everything core-dependent arrives as
per-core ExternalInput data (the mask slice in local-t coordinates, an
edge mask for the conv zero-padding, and a per-core output slice), so all
cores share one SPMD instruction stream.

Weights are baked into the NEFF as inline consts; the per-core mask slice
(4.5MB vs the full 64MB) is a per-core input.
"""

import hashlib
import os
import sys

import numpy as np

for _p in ("/opt/trn_rl_repo", "/root/.axon_site/_ro/trn_rl_repo"):
    if _p not in sys.path:
        sys.path.append(_p)

import ml_dtypes  # noqa: E402
import concourse.bass as bass  # noqa: E402
import concourse.bacc as bacc  # noqa: E402
import concourse.tile as tile  # noqa: E402
import concourse.mybir as mybir  # noqa: E402
from contextlib import ExitStack  # noqa: E402
from concourse.masks import make_identity  # noqa: E402

F32 = mybir.dt.float32
BF16 = mybir.dt.bfloat16
AF = mybir.ActivationFunctionType
BFNP = ml_dtypes.bfloat16

B = 2
DIM = 512
T = 256
H1 = 256
H3 = 512
H2 = 128
N = 32
W = 8
NCORES = 8
EPS = 1e-5
TC = 32            # owned t's per core
TL = TC + 2        # local window incl halo
OC = TL * W        # 272 sampling cols per (n, b)
OWN_LO, OWN_HI = W, W + TC * W   # owned col range [8, 264) within OC

# rows of the packed per-channel vector table (same layout as v2)
V_C1B = 0
V_GN1G = 2
V_GN1B = 4
V_R3DB = 6
V_GN3G = 10
V_GN3B = 14
V_R2DB = 18
V_GN2G = 19
V_GN2B = 20
V_S1B = 21
V_E1B = 22
V_SGNG = 23
V_SGNB = 24
V_EGNG = 25
V_EGNB = 26
V_S2B = 27
V_E2B = 28
V_BG3G = 29
V_BG3B = 37
V_BG3C = 45
V_BG2G = 53
V_BG2B = 55
V_BG2C = 57
V_BHG = 59
V_BHB = 63
V_BHC = 67
NVEC = 71

WT_R2D = 0
WT_S1 = 4
WT_E1 = 13
WT_S2 = 22


def _build(consts, reps=1):
    nc = bacc.Bacc("TRN2", target_bir_lowering=False, debug=False, num_devices=NCORES)

    xin = nc.declare_dram_parameter("x_in", [B, DIM, T], BF16, isOutput=False)
    mpin = nc.declare_dram_parameter("maskp", [2, 128, N, OC], BF16, isOutput=False)
    edgein = nc.declare_dram_parameter("edge", [128, 2], F32, isOutput=False)
    outd = nc.declare_dram_parameter("out", [B, 2, TC, W], F32, isOutput=True)

    c1w = nc.inline_tensor(consts["c1w"], name="c1w")
    r3dw = nc.inline_tensor(consts["r3dw"], name="r3dw")
    wtail = nc.inline_tensor(consts["wtail"], name="wtail")
    gmats = nc.inline_tensor(consts["gmats"], name="gmats")
    emats = nc.inline_tensor(consts["emats"], name="emats")
    vecsd = nc.inline_tensor(consts["vecs"], name="vecs")

    with tile.TileContext(nc) as tc, ExitStack() as ctx:
        consts_p = ctx.enter_context(tc.tile_pool(name="consts", bufs=1))
        bigres = ctx.enter_context(tc.tile_pool(name="bigres", bufs=1))
        small = ctx.enter_context(tc.tile_pool(name="small", bufs=8))
        dram = ctx.enter_context(tc.tile_pool(name="dram", bufs=1, space="DRAM"))

        for _rep in range(reps):
            _body(nc, tc, consts_p, bigres, small, dram,
                  xin, mpin, edgein, outd, c1w, r3dw, wtail, gmats, emats, vecsd)

    nc.compile()
    return nc


def _body(nc, tc, consts_p, bigres, small, dram,
          xin, mpin, edgein, outd, c1w, r3dw, wtail, gmats, emats, vecsd):
    # ---- loads ----
    r3d_sb = bigres.tile([128, 64, H3], BF16, name="r3d", tag="r3d")
    for _ci in range(8):
        nc.scalar.dma_start(out=r3d_sb[:, _ci * 4:(_ci + 1) * 4, :],
                            in_=r3dw[:, _ci * 4:(_ci + 1) * 4, :])
    for _ci in range(8, 16):
        nc.gpsimd.dma_start(out=r3d_sb[:, _ci * 4:(_ci + 1) * 4, :],
                            in_=r3dw[:, _ci * 4:(_ci + 1) * 4, :])

    mp_sb = bigres.tile([128, 2, N, OC], BF16, name="mp", tag="mp")
    for tt in range(2):
        for nh in range(2):
            nc.sync.dma_start(out=mp_sb[:, tt, nh * 16:(nh + 1) * 16, :],
                              in_=mpin[tt, :, nh * 16:(nh + 1) * 16, :])

    x_sb = bigres.tile([128, 4, B, T + 2], BF16, name="xsb", tag="xsb")
    nc.vector.memset(x_sb[:, :, :, 0:1], 0.0)
    nc.vector.memset(x_sb[:, :, :, T + 1:T + 2], 0.0)
    for b in range(B):
        nc.sync.dma_start(
            out=x_sb[:, :, b, 1:T + 1],
            in_=bass.AP(tensor=xin, offset=b * DIM * T,
                        ap=[[T, 128], [128 * T, 4], [1, T]]))
    c1w_sb = consts_p.tile([128, 12, H1], BF16)
    nc.sync.dma_start(out=c1w_sb, in_=c1w[:, :, :])
    vec_sb = consts_p.tile([128, NVEC_TOTAL], F32)
    nc.sync.dma_start(out=vec_sb, in_=bass.AP(tensor=vecsd, offset=0, ap=[[1, 128], [128, NVEC_TOTAL]]))
    gm_sb = consts_p.tile([128, 56], F32)
    nc.sync.dma_start(out=gm_sb, in_=gmats[:, :])
    e8_sb = consts_p.tile([16, 128], F32)
    nc.sync.dma_start(out=e8_sb, in_=emats[0:16, :])
    e16_sb = consts_p.tile([8, 128], F32)
    nc.sync.dma_start(out=e16_sb, in_=emats[32:40, :])
    e4_sb = consts_p.tile([32, 128], F32)
    nc.sync.dma_start(out=e4_sb, in_=emats[64:96, :])
    wt_sb = consts_p.tile([128, 23, H2], BF16)
    nc.sync.dma_start(out=wt_sb, in_=wtail[:, :, :])
    edge_sb = bigres.tile([128, 2], F32, name="edge", tag="edge")
    nc.sync.dma_start(out=edge_sb, in_=edgein[:, :])

    g8_sb = gm_sb[:, 0:16]
    g16_sb = gm_sb[:, 16:24]
    g4_sb = gm_sb[:, 24:56]
    r2d_sb = wt_sb[:, WT_R2D:WT_R2D + 4, :]
    s1w_sb = wt_sb[:, WT_S1:WT_S1 + 9, :]
    e1w_sb = wt_sb[:, WT_E1:WT_E1 + 9, :]
    s2w_sb = wt_sb[:, WT_S2, 0:2]

    epsT = consts_p.tile([32, 1], F32)
    nc.vector.memset(epsT, EPS)
    ident = consts_p.tile([128, 128], F32)
    make_identity(nc, ident)

    def vcol(r):
        return vec_sb[:, r:r + 1]

    # ---- GroupNorm helpers ----
    def stats_from(pstat, src_ap, bias_ap, G, gdim, dst):
        cols = src_ap.free_size()
        assert cols <= 512
        st6 = small.tile([128, 4, 6], F32, name="st6", tag="st6")[:, :1, :]
        nc.vector.bn_stats(out=st6, in_=src_ap)
        stats_tail(pstat, st6, bias_ap, G, gdim, dst)

    def stats_tail(pstat, st6_ap, bias_ap, G, gdim, dst):
        mv = small.tile([128, 2], F32, name="mv", tag="mv")
        nc.vector.bn_aggr(out=mv, in_=st6_ap)
        s12 = small.tile([128, 2], F32, name="s12", tag="s12")
        nc.vector.tensor_scalar_add(s12[:, 0:1], mv[:, 0:1], bias_ap)
        sq = small.tile([128, 1], F32, name="sq", tag="sq")
        nc.vector.tensor_mul(sq, s12[:, 0:1], s12[:, 0:1])
        nc.vector.tensor_add(s12[:, 1:2], mv[:, 1:2], sq)
        pg = pstat.tile([gdim, 2], F32, name="pst", tag="pst")
        nc.tensor.matmul(pg, G[:, :], s12, start=True, stop=True)
        nc.vector.tensor_copy(dst, pg)

    def gn_prep(pstat, E, gdim, stg_view, ni, gG, gB, gC):
        """Batched finalize from group stats (already global): per-channel
        (scale, bias) for ni instances."""
        rm = small.tile([32, 8, 2], F32, name="rmb", tag="rmb")[:gdim, :ni, :]
        sq = small.tile([32, 8], F32, name="sqb", tag="sqb")[:gdim, :ni]
        var = small.tile([32, 8], F32, name="varb", tag="varb")[:gdim, :ni]
        nc.vector.tensor_mul(sq, stg_view[:, :, 0], stg_view[:, :, 0])
        nc.vector.tensor_sub(var, stg_view[:, :, 1], sq)
        nc.scalar.activation(out=var, in_=var, func=AF.Sqrt, bias=epsT[:gdim], scale=1.0)
        nc.vector.reciprocal(rm[:, :, 0], var)
        nc.vector.tensor_copy(rm[:, :, 1], stg_view[:, :, 0])
        pb = pstat.tile([128, 8, 2], F32, name="pstb", tag="pstb")[:, :ni, :]
        nc.tensor.matmul(pb, E[:, :], rm, start=True, stop=True)
        scale = small.tile([128, 8], F32, name="scaleb", tag="scaleb", bufs=2)[:, :ni]
        bias = small.tile([128, 8], F32, name="biasb", tag="biasb", bufs=2)[:, :ni]
        t1 = small.tile([128, 8], F32, name="t1b", tag="t1b")[:, :ni]
        nc.vector.tensor_mul(scale, pb[:, :, 0], vec_sb[:, gG:gG + ni])
        nc.vector.tensor_sub(t1, vec_sb[:, gC:gC + ni], pb[:, :, 1])
        nc.vector.tensor_mul(t1, t1, scale)
        nc.vector.tensor_add(bias, t1, vec_sb[:, gB:gB + ni])
        return scale, bias

    def cc_warm(tag):
        """Dummy tiny AllReduce issued ahead of a real sync point, hidden
        under compute, to keep the collective pipeline warm."""
        if not os.environ.get("KB3_CCWARM"):
            return
        wb = dram.tile([8, 4], F32, name=f"wb_{tag}", tag=f"wb_{tag}")
        wo = dram.tile([8, 4], F32, name=f"wo_{tag}", tag=f"wo_{tag}")
        ws = bigres.tile([8, 4], F32, name=f"ws_{tag}", tag=f"ws_{tag}")
        nc.gpsimd.memset(ws, 1.0)
        nc.gpsimd.dma_start(wb[:, :], ws)
        nc.gpsimd.collective_compute(
            "AllReduce", mybir.AluOpType.add,
            replica_groups=[list(range(NCORES))],
            ins=[wb.opt()], outs=[wo.opt()])
        nc.gpsimd.dma_start(ws, wo[:, :])

    def cc_average(st_tile, flat_parts, flat_cols, tag):
        """AllReduce-add the partial stats tile across the 8 cores, then
        scale by 1/8 (equal per-core element counts -> global mean)."""
        flat = st_tile.rearrange(f"p a b -> p (a b)") if len(st_tile.shape) == 3 else \
            st_tile.rearrange(f"p a b c -> p (a b c)")
        if os.environ.get("KB3_NOCC"):
            # timing-only variant: skip the cross-core exchange (output WRONG)
            nc.vector.tensor_scalar_mul(flat, flat, 1.0)
            return
        if os.environ.get("KB3_CC1ONLY") and tag != "g3":
            nc.vector.tensor_scalar_mul(flat, flat, 1.0)
            return
        ib = dram.tile([flat_parts, flat_cols], F32, name=f"ib_{tag}", tag=f"ib_{tag}")
        ob = dram.tile([flat_parts, flat_cols], F32, name=f"ob_{tag}", tag=f"ob_{tag}")
        nc.gpsimd.dma_start(ib[:, :], flat)
        nc.gpsimd.collective_compute(
            "AllReduce", mybir.AluOpType.add,
            replica_groups=[list(range(NCORES))],
            ins=[ib.opt()], outs=[ob.opt()])
        nc.gpsimd.dma_start(flat, ob[:, :])
        nc.vector.tensor_scalar_mul(flat, flat, 1.0 / NCORES)

    # ---- stage 1: conv1 + GN1 + ReLU + transpose (full T, redundant) ----
    h_sb = [[bigres.tile([128, T], F32, name=f"h{b}{mt}", tag=f"h{b}{mt}") for mt in range(2)] for b in range(B)]
    hT_sb = [[bigres.tile([128, H1], BF16, name=f"ht{b}{tt}", tag=f"ht{b}{tt}") for tt in range(2)] for b in range(B)]
    st1 = bigres.tile([16, B, 2, 2], F32, name="st1", tag="st1")

    with tc.tile_pool(name="ps1", bufs=1, space="PSUM") as ps1:
        ph = {}
        for mt in range(2):
            ph[mt] = ps1.tile([128, B, T], F32, name="ph", tag=f"ph{mt}")
            for idx in range(12):
                j, ct = idx // 4, idx % 4
                nc.tensor.matmul(
                    ph[mt],
                    c1w_sb[:, idx, mt * 128:(mt + 1) * 128],
                    x_sb[:, ct, :, j:j + T],
                    start=(idx == 0), stop=(idx == 11),
                )
            for b in range(B):
                # stage-1 stats are full-T and per-core exact (no collective)
                cols = T
                st6 = small.tile([128, 4, 6], F32, name="st6", tag="st6")[:, :1, :]
                nc.vector.bn_stats(out=st6, in_=ph[mt][:, b, :])
                stats_tail(ps1, st6, vcol(V_C1B + mt), g8_sb, 16, st1[:, b, mt, :])
        sc1, bi1 = gn_prep(ps1, e8_sb, 16,
                           st1.rearrange("g b m s -> g (b m) s"), 4,
                           V_BG1G, V_BG1B, V_BG1C)
        for b in range(B):
            for mt in range(2):
                i = b * 2 + mt
                nc.scalar.activation(out=h_sb[b][mt], in_=ph[mt][:, b, :],
                                     func=AF.Relu, bias=sc_bias_col(bi1, i), scale=sc_bias_col(sc1, i))
            for tt in range(2):
                for mt in range(2):
                    pt = ps1.tile([128, 128], F32, name="pt", tag="pt", bufs=2)
                    nc.tensor.transpose(pt, h_sb[b][mt][:, tt * 128:(tt + 1) * 128], ident)
                    nc.vector.tensor_copy(hT_sb[b][tt][:, mt * 128:(mt + 1) * 128], pt)

    # ---- stage 2': sampling GEMM on the core-local window ----
    ssamp = [bigres.tile([128, N, B, OC], BF16, name=f"ss{ct}", tag=f"ss{ct}") for ct in range(2)]
    eng_rot = [0]

    def rot_copy(dst, src):
        # PSUM -> SBUF: only Act (scalar) and DVE (vector) can read PSUM
        e = eng_rot[0] % 2
        eng_rot[0] += 1
        if e == 0:
            nc.scalar.activation(out=dst, in_=src, func=AF.Copy)
        else:
            nc.vector.tensor_copy(dst, src)

    # stage 2+3 interleaved: sampling for sample n feeds the Conv3d
    # accumulation for k=2n,2n+1 while sampling for n+1 proceeds.
    # py holds only the owned 256 cols (1 PSUM bank per ot); the 16 halo
    # cols accumulate in a separate tiny pass afterwards.
    y_sb = [bigres.tile([128, B, OC], BF16, name=f"y{ot}", tag=f"y{ot}") for ot in range(4)]
    st3 = bigres.tile([8, B, 4, 2], F32, name="st3", tag="st3")
    st2 = bigres.tile([32, B, 2], F32, name="st2", tag="st2")
    sth = bigres.tile([32, B, 2, 2], F32, name="sth", tag="sth")

    # stage 3 runs in two ot-waves of half-width (136-col) PSUM tiles so the
    # full 272-col window accumulates in-bank; wave 0 (ot 0,1) interleaves
    # with the sampling loop, wave 1 (ot 2,3) streams right after.
    HC = OC // 2  # 136

    def wave_matmuls(ot_pair, k, pyt):
        n, ct = k >> 1, k & 1
        for oi, ot in enumerate(ot_pair):
            for hh in range(2):
                nc.tensor.matmul(
                    pyt[oi * 2 + hh],
                    r3d_sb[:, k, ot * 128:(ot + 1) * 128],
                    ssamp[ct][:, n, :, hh * HC:(hh + 1) * HC],
                    start=(k == 0), stop=(k == 63),
                )

    with tc.tile_pool(name="ps23", bufs=1, space="PSUM") as ps23:
        pyw = [ps23.tile([128, B, HC], F32, name="pyw", tag=f"pyw{i}") for i in range(4)]
        for n in range(N):
            for ct in range(2):
                for b in range(B):
                    ps2 = ps23.tile([128, OC], F32, name="ps2", tag="ps2", bufs=3)
                    for tt in range(2):
                        nc.tensor.matmul(
                            ps2,
                            hT_sb[b][tt][:, ct * 128:(ct + 1) * 128],
                            mp_sb[:, tt, n, :],
                            start=(tt == 0), stop=(tt == 1),
                        )
                    rot_copy(ssamp[ct][:, n, b, :], ps2)
            for ct in range(2):
                wave_matmuls((0, 1), n * 2 + ct, pyw)
        for i, (ot, hh) in enumerate([(o, h) for o in (0, 1) for h in range(2)]):
            for b in range(B):
                rot_copy(y_sb[ot][:, b, hh * HC:(hh + 1) * HC], pyw[i][:, b, :])

    cc_warm("w1")
    with tc.tile_pool(name="ps3b", bufs=1, space="PSUM") as ps3b:
        pyw2 = [ps3b.tile([128, B, HC], F32, name="pyw2", tag=f"pyw2{i}") for i in range(4)]
        for k in range(64):
            wave_matmuls((2, 3), k, pyw2)
        for i, (ot, hh) in enumerate([(o, h) for o in (2, 3) for h in range(2)]):
            for b in range(B):
                rot_copy(y_sb[ot][:, b, hh * HC:(hh + 1) * HC], pyw2[i][:, b, :])

    fpre = bigres.tile([128, B, OC], BF16, name="fpre", tag="fpre")
    f_sb = [bigres.tile([128, TL + 2, W + 2], BF16, name=f"f{b}", tag=f"f{b}") for b in range(B)]
    hpre = [bigres.tile([128, TC * W], BF16, name=f"hpre{i}", tag=f"hpre{i}") for i in range(4)]

    with tc.tile_pool(name="ps45", bufs=1, space="PSUM") as ps45:
        for ot in range(4):
            for b in range(B):
                stats_from(ps45, y_sb[ot][:, b, OWN_LO:OWN_HI], vcol(V_R3DB + ot),
                           g16_sb, 8, st3[:, b, ot, :])

        # ---- collective 1 + GN3 prep ----
        cc_average(st3, 8, 16, "g3")
        cc_warm("w2")
        sc3, bi3 = gn_prep(ps45, e16_sb, 8, st3.rearrange("g b o s -> g (b o) s"), 8,
                           V_BG3G, V_BG3B, V_BG3C)
        for b in range(B):
            for ot in range(4):
                i = b * 4 + ot
                nc.scalar.activation(out=y_sb[ot][:, b, :], in_=y_sb[ot][:, b, :],
                                     func=AF.Relu, bias=bi3[:, i:i + 1], scale=sc3[:, i:i + 1])
            pf = ps45.tile([128, OC], F32, name="pf", tag="pf", bufs=2)
            for ot in range(4):
                nc.tensor.matmul(pf, r2d_sb[:, ot, :], y_sb[ot][:, b, :],
                                 start=(ot == 0), stop=(ot == 3))
            rot_copy(fpre[:, b, :], pf)
            stats_from(ps45, fpre[:, b, OWN_LO:OWN_HI], vcol(V_R2DB), g4_sb, 32, st2[:, b, :])
            nc.vector.memset(f_sb[b], 0.0)

        # ---- collective 2 + GN2 prep/apply ----
        cc_average(st2, 32, 2 * B, "g2")
        cc_warm("w3")
        sc2, bi2 = gn_prep(ps45, e4_sb, 32, st2, 2, V_BG2G, V_BG2B, V_BG2C)
        for b in range(B):
            nc.scalar.activation(out=f_sb[b][:, 1:TL + 1, 1:W + 1], in_=fpre[:, b, :],
                                 func=AF.Relu, bias=bi2[:, b:b + 1], scale=sc2[:, b:b + 1])
            # conv zero-padding at the global t boundary: the halo columns are
            # real data on interior cores, zero on edge cores (edge in {0,1}).
            for side in range(2):
                scE = small.tile([128, 1], F32, name="scE", tag="scE", bufs=2)
                biE = small.tile([128, 1], F32, name="biE", tag="biE", bufs=2)
                nc.vector.tensor_mul(scE, sc2[:, b:b + 1], edge_sb[:, side:side + 1])
                nc.vector.tensor_mul(biE, bi2[:, b:b + 1], edge_sb[:, side:side + 1])
                lt = 0 if side == 0 else TL - 1
                nc.scalar.activation(out=f_sb[b][:, lt + 1:lt + 2, 1:W + 1],
                                     in_=fpre[:, b, lt * W:(lt + 1) * W],
                                     func=AF.Relu, bias=biE, scale=scE)

        # ---- stage 5: heads ----
        for b in range(B):
            for hd in range(2):
                w_sb = s1w_sb if hd == 0 else e1w_sb
                i = b * 2 + hd
                phd = ps45.tile([128, TC * W], F32, name="phd", tag="phd", bufs=2)
                for tap in range(9):
                    kt, kw = tap // 3, tap % 3
                    # out t' in [0,32) maps to f_sb col (t'+1+kt, kw) in the
                    # (TL+2, W+2) padded layout
                    nc.tensor.matmul(phd, w_sb[:, tap, :],
                                     f_sb[b][:, 1 + kt:1 + kt + TC, kw:kw + W],
                                     start=(tap == 0), stop=(tap == 8))
                rot_copy(hpre[i], phd)
                stats_from(ps45, hpre[i], vcol(V_S1B + hd), g4_sb, 32, sth[:, b, hd, :])

        # ---- collective 3 + head GN + final 1x1 + sigmoid ----
        cc_average(sth, 32, 2 * B * 2, "gh")
        sch, bih = gn_prep(ps45, e4_sb, 32, sth.rearrange("g b h s -> g (b h) s"), 4,
                           V_BHG, V_BHB, V_BHC)
        for b in range(B):
            for hd in range(2):
                i = b * 2 + hd
                nc.scalar.activation(out=hpre[i], in_=hpre[i], func=AF.Relu,
                                     bias=bih[:, i:i + 1], scale=sch[:, i:i + 1])
                po = ps45.tile([1, TC * W], F32, name="po", tag="po", bufs=2)
                nc.tensor.matmul(po, s2w_sb[:, hd:hd + 1], hpre[i], start=True, stop=True)
                o_one = bigres.tile([1, TC * W], F32, name="o_one", tag="o_one", bufs=2)
                nc.scalar.activation(out=o_one, in_=po, func=AF.Sigmoid,
                                     bias=vec_sb[0:1, V_S2B + hd:V_S2B + hd + 1], scale=1.0)
                nc.sync.dma_start(out=outd[b, hd, :, :], in_=o_one)


# stage-1 batched GN prep needs bg1 vec rows; extend the table
V_BG1G = NVEC
V_BG1B = NVEC + 4
V_BG1C = NVEC + 8
NVEC_TOTAL = NVEC + 12


def sc_bias_col(mat, i):
    return mat[:, i:i + 1]


def _f32(a):
    return np.ascontiguousarray(np.asarray(a, dtype=np.float32))


def _prep_consts(inputs):
    c1_w = _f32(inputs["c1_w"])
    r3d_w = _f32(inputs["r3d_w"])[:, :, :, 0, 0]
    r2d_w = _f32(inputs["r2d_w"])[:, :, 0, 0]
    s1_w = _f32(inputs["s1_w"])
    e1_w = _f32(inputs["e1_w"])
    s2_w = _f32(inputs["s2_w"])[0, :, 0, 0]
    e2_w = _f32(inputs["e2_w"])[0, :, 0, 0]

    # conv1 weights: [c, j*4+ct, m] = c1_w[m, ct*128+c, j]
    a = c1_w.transpose(1, 2, 0).reshape(4, 128, 3, H1)
    c1w_h = np.ascontiguousarray(a.transpose(1, 2, 0, 3).reshape(128, 12, H1)).astype(BFNP)

    # r3d weights: [c, n*2+ct, o] = r3d_w[o, ct*128+c, n]
    a = r3d_w.transpose(1, 2, 0).reshape(2, 128, N, H3)
    r3d_h = np.ascontiguousarray(a.transpose(1, 2, 0, 3).reshape(128, 64, H3)).astype(BFNP)

    wtail = np.zeros((128, 23, H2), np.float32)
    wtail[:, WT_R2D:WT_R2D + 4, :] = r2d_w.T.reshape(4, 128, H2).transpose(1, 0, 2)
    wtail[:, WT_S1:WT_S1 + 9, :] = s1_w.transpose(1, 2, 3, 0).reshape(128, 9, H2)
    wtail[:, WT_E1:WT_E1 + 9, :] = e1_w.transpose(1, 2, 3, 0).reshape(128, 9, H2)
    wtail[:, WT_S2, 0] = s2_w
    wtail[:, WT_S2, 1] = e2_w
    wtail_h = wtail.astype(BFNP)

    ch = np.arange(128)
    g8 = (ch[:, None] // 8 == np.arange(16)[None, :]).astype(np.float32)
    g16 = (ch[:, None] // 16 == np.arange(8)[None, :]).astype(np.float32)
    g4 = (ch[:, None] // 4 == np.arange(32)[None, :]).astype(np.float32)
    gmats = np.concatenate([g8 / 8.0, g16 / 16.0, g4 / 4.0], axis=1)
    emats = np.zeros((96, 128), np.float32)
    emats[0:16] = g8.T
    emats[32:40] = g16.T
    emats[64:96] = g4.T

    vecs = np.zeros((NVEC_TOTAL, 128), np.float32)
    vecs[V_C1B:V_C1B + 2] = _f32(inputs["c1_b"]).reshape(2, 128)
    vecs[V_GN1G:V_GN1G + 2] = _f32(inputs["gn1_g"]).reshape(2, 128)
    vecs[V_GN1B:V_GN1B + 2] = _f32(inputs["gn1_b"]).reshape(2, 128)
    vecs[V_R3DB:V_R3DB + 4] = _f32(inputs["r3d_b"]).reshape(4, 128)
    vecs[V_GN3G:V_GN3G + 4] = _f32(inputs["gn3_g"]).reshape(4, 128)
    vecs[V_GN3B:V_GN3B + 4] = _f32(inputs["gn3_b"]).reshape(4, 128)
    vecs[V_R2DB] = _f32(inputs["r2d_b"])
    vecs[V_GN2G] = _f32(inputs["gn2_g"])
    vecs[V_GN2B] = _f32(inputs["gn2_b"])
    vecs[V_S1B] = _f32(inputs["s1_b"])
    vecs[V_E1B] = _f32(inputs["e1_b"])
    vecs[V_SGNG] = _f32(inputs["sgn_g"])
    vecs[V_SGNB] = _f32(inputs["sgn_b"])
    vecs[V_EGNG] = _f32(inputs["egn_g"])
    vecs[V_EGNB] = _f32(inputs["egn_b"])
    vecs[V_S2B] = _f32(inputs["s2_b"])[0]
    vecs[V_E2B] = _f32(inputs["e2_b"])[0]
    gn3g4 = _f32(inputs["gn3_g"]).reshape(4, 128)
    gn3b4 = _f32(inputs["gn3_b"]).reshape(4, 128)
    r3db4 = _f32(inputs["r3d_b"]).reshape(4, 128)
    for i, (b, ot) in enumerate([(b, ot) for b in range(B) for ot in range(4)]):
        vecs[V_BG3G + i] = gn3g4[ot]
        vecs[V_BG3B + i] = gn3b4[ot]
        vecs[V_BG3C + i] = r3db4[ot]
    for b in range(B):
        vecs[V_BG2G + b] = _f32(inputs["gn2_g"])
        vecs[V_BG2B + b] = _f32(inputs["gn2_b"])
        vecs[V_BG2C + b] = _f32(inputs["r2d_b"])
    hg = [_f32(inputs["sgn_g"]), _f32(inputs["egn_g"])]
    hb = [_f32(inputs["sgn_b"]), _f32(inputs["egn_b"])]
    hc = [_f32(inputs["s1_b"]), _f32(inputs["e1_b"])]
    for i, (b, hd) in enumerate([(b, hd) for b in range(B) for hd in range(2)]):
        vecs[V_BHG + i] = hg[hd]
        vecs[V_BHB + i] = hb[hd]
        vecs[V_BHC + i] = hc[hd]
    gn1g2 = _f32(inputs["gn1_g"]).reshape(2, 128)
    gn1b2 = _f32(inputs["gn1_b"]).reshape(2, 128)
    c1b2 = _f32(inputs["c1_b"]).reshape(2, 128)
    for i, (b, mt) in enumerate([(b, mt) for b in range(B) for mt in range(2)]):
        vecs[V_BG1G + i] = gn1g2[mt]
        vecs[V_BG1B + i] = gn1b2[mt]
        vecs[V_BG1C + i] = c1b2[mt]

    return {
        "c1w": c1w_h, "r3dw": r3d_h, "wtail": wtail_h,
        "gmats": gmats, "emats": emats, "vecs": vecs,
        "mask": _f32(inputs["sample_mask"]).reshape(T, N, T, W),
    }


def _fingerprint(inputs):
    h = hashlib.sha1()
    for k in sorted(inputs.keys()):
        if k == "x":
            continue
        a = np.asarray(inputs[k])
        h.update(k.encode())
        h.update(str(a.shape).encode())
        h.update(str(a.dtype).encode())
        flat = a.reshape(-1)
        step = max(1, flat.size // 65536)
        h.update(np.ascontiguousarray(flat[::step]).tobytes())
    return h.hexdigest()


_module_cache = {}


def _get_module(inputs=None):
    if inputs is None:
        if "nc" not in _module_cache:
            raise RuntimeError("module not built yet; call kernel() first")
        return _module_cache["nc"]
    fp = _fingerprint(inputs)
    if _module_cache.get("fp") != fp:
        consts = _prep_consts(inputs)
        _module_cache["nc"] = _build(consts)
        _module_cache["fp"] = fp
        _module_cache["consts"] = consts
    return _module_cache["nc"]


def _prep(inputs):
    """Per-core input maps: full x, per-core mask slice + edge mask."""
    x_h = np.ascontiguousarray(_f32(inputs["x"]).astype(BFNP))
    mask = _f32(inputs["sample_mask"]).reshape(T, N, T, W)
    maps = []
    for c in range(NCORES):
        mp = np.zeros((2, 128, N, OC), np.float32)
        for lt in range(TL):
            gt = TC * c - 1 + lt
            if 0 <= gt < T:
                # mp[tt, p, n, lt*W+w] = mask[tt*128+p, n, gt, w]
                mslice = mask[:, :, gt, :]  # (T', N, W)
                mp[:, :, :, lt * W:(lt + 1) * W] = mslice.reshape(2, 128, N, W)
        edge = np.ones((128, 2), np.float32)
        if c == 0:
            edge[:, 0] = 0.0
        if c == NCORES - 1:
            edge[:, 1] = 0.0
        maps.append({
            "x_in": x_h,
            "maskp": np.ascontiguousarray(mp.astype(BFNP)),
            "edge": edge,
        })
    return maps


def kernel(**inputs) -> np.ndarray:
    nc = _get_module(inputs)
    in_maps = _prep(inputs)
    from concourse.bass_utils import run_bass_kernel_spmd
    res = run_bass_kernel_spmd(nc, in_maps, list(range(NCORES)))
    full = np.zeros((B, 2, T, W), np.float32)
    for c in range(NCORES):
        full[:, :, TC * c:TC * (c + 1), :] = res.results[c]["out"].astype(np.float32)
    return np.ascontiguousarray(full)


# revision 21
# speedup vs baseline: 1.0230x; 1.0230x over previous
"""BEM (boundary evaluation module) Trainium2 kernel, v3: t-sharded SPMD.

Each of the 8 cores owns a 32-wide slice of the T=256 axis.  Stage 1
(conv1+GN1, ~2% of the FLOPs) is computed redundantly on every core; the
expensive sampling GEMM and Conv3d reduction (stages 2-3) and the whole
tail run only on each core's 34-wide haloed window (+-1 t for the heads'
3x3 convs).  GroupNorm statistics are global over (T, W), so the three GN
points after stage 3 exchange per-core partial moments with tiny (<=1KB)
AllReduces (~1-2us each measured).  Everything core-dependent arrives as
per-core ExternalInput data (the mask slice in local-t coordinates, an
edge mask for the conv zero-padding, and a per-core output slice), so all
cores share one SPMD instruction stream.

Weights are baked into the NEFF as inline consts; the per-core mask slice
(4.5MB vs the full 64MB) is a per-core input.
"""

import hashlib
import os
import sys

import numpy as np

for _p in ("/opt/trn_rl_repo", "/root/.axon_site/_ro/trn_rl_repo"):
    if _p not in sys.path:
        sys.path.append(_p)

import ml_dtypes  # noqa: E402
import concourse.bass as bass  # noqa: E402
import concourse.bacc as bacc  # noqa: E402
import concourse.tile as tile  # noqa: E402
import concourse.mybir as mybir  # noqa: E402
from contextlib import ExitStack  # noqa: E402
from concourse.masks import make_identity  # noqa: E402

F32 = mybir.dt.float32
BF16 = mybir.dt.bfloat16
AF = mybir.ActivationFunctionType
BFNP = ml_dtypes.bfloat16

B = 2
DIM = 512
T = 256
H1 = 256
H3 = 512
H2 = 128
N = 32
W = 8
NCORES = 8
EPS = 1e-5
TC = 32            # owned t's per core
TL = TC + 2        # local window incl halo
OC = TL * W        # 272 sampling cols per (n, b)
OWN_LO, OWN_HI = W, W + TC * W   # owned col range [8, 264) within OC

# rows of the packed per-channel vector table (same layout as v2)
V_C1B = 0
V_GN1G = 2
V_GN1B = 4
V_R3DB = 6
V_GN3G = 10
V_GN3B = 14
V_R2DB = 18
V_GN2G = 19
V_GN2B = 20
V_S1B = 21
V_E1B = 22
V_SGNG = 23
V_SGNB = 24
V_EGNG = 25
V_EGNB = 26
V_S2B = 27
V_E2B = 28
V_BG3G = 29
V_BG3B = 37
V_BG3C = 45
V_BG2G = 53
V_BG2B = 55
V_BG2C = 57
V_BHG = 59
V_BHB = 63
V_BHC = 67
NVEC = 71

WT_R2D = 0
WT_S1 = 4
WT_E1 = 13
WT_S2 = 22


def _build(consts, reps=1):
    nc = bacc.Bacc("TRN2", target_bir_lowering=False, debug=False, num_devices=NCORES)

    xin = nc.declare_dram_parameter("x_in", [B, DIM, T], BF16, isOutput=False)
    mpin = nc.declare_dram_parameter("maskp", [2, 128, N, OC], BF16, isOutput=False)
    edgein = nc.declare_dram_parameter("edge", [128, 2], F32, isOutput=False)
    outd = nc.declare_dram_parameter("out", [B, 2, TC, W], F32, isOutput=True)

    c1w = nc.inline_tensor(consts["c1w"], name="c1w")
    r3dw = nc.inline_tensor(consts["r3dw"], name="r3dw")
    wtail = nc.inline_tensor(consts["wtail"], name="wtail")
    gmats = nc.inline_tensor(consts["gmats"], name="gmats")
    emats = nc.inline_tensor(consts["emats"], name="emats")
    vecsd = nc.inline_tensor(consts["vecs"], name="vecs")

    with tile.TileContext(nc) as tc, ExitStack() as ctx:
        consts_p = ctx.enter_context(tc.tile_pool(name="consts", bufs=1))
        bigres = ctx.enter_context(tc.tile_pool(name="bigres", bufs=1))
        small = ctx.enter_context(tc.tile_pool(name="small", bufs=8))
        dram = ctx.enter_context(tc.tile_pool(name="dram", bufs=1, space="DRAM"))

        for _rep in range(reps):
            _body(nc, tc, consts_p, bigres, small, dram,
                  xin, mpin, edgein, outd, c1w, r3dw, wtail, gmats, emats, vecsd)

    nc.compile()
    return nc


def _body(nc, tc, consts_p, bigres, small, dram,
          xin, mpin, edgein, outd, c1w, r3dw, wtail, gmats, emats, vecsd):
    # ---- loads ----
    r3d_sb = bigres.tile([128, 64, H3], BF16, name="r3d", tag="r3d")
    for _ci in range(8):
        nc.scalar.dma_start(out=r3d_sb[:, _ci * 4:(_ci + 1) * 4, :],
                            in_=r3dw[:, _ci * 4:(_ci + 1) * 4, :])
    for _ci in range(8, 16):
        nc.gpsimd.dma_start(out=r3d_sb[:, _ci * 4:(_ci + 1) * 4, :],
                            in_=r3dw[:, _ci * 4:(_ci + 1) * 4, :])

    mp_sb = bigres.tile([128, 2, N, OC], BF16, name="mp", tag="mp")
    for tt in range(2):
        for nh in range(2):
            nc.sync.dma_start(out=mp_sb[:, tt, nh * 16:(nh + 1) * 16, :],
                              in_=mpin[tt, :, nh * 16:(nh + 1) * 16, :])

    x_sb = bigres.tile([128, 4, B, T + 2], BF16, name="xsb", tag="xsb")
    nc.vector.memset(x_sb[:, :, :, 0:1], 0.0)
    nc.vector.memset(x_sb[:, :, :, T + 1:T + 2], 0.0)
    for b in range(B):
        nc.sync.dma_start(
            out=x_sb[:, :, b, 1:T + 1],
            in_=bass.AP(tensor=xin, offset=b * DIM * T,
                        ap=[[T, 128], [128 * T, 4], [1, T]]))
    c1w_sb = consts_p.tile([128, 12, H1], BF16)
    nc.sync.dma_start(out=c1w_sb, in_=c1w[:, :, :])
    vec_sb = consts_p.tile([128, NVEC_TOTAL], F32)
    nc.sync.dma_start(out=vec_sb, in_=bass.AP(tensor=vecsd, offset=0, ap=[[1, 128], [128, NVEC_TOTAL]]))
    gm_sb = consts_p.tile([128, 56], F32)
    nc.sync.dma_start(out=gm_sb, in_=gmats[:, :])
    e8_sb = consts_p.tile([16, 128], F32)
    nc.sync.dma_start(out=e8_sb, in_=emats[0:16, :])
    e16_sb = consts_p.tile([8, 128], F32)
    nc.sync.dma_start(out=e16_sb, in_=emats[32:40, :])
    e4_sb = consts_p.tile([32, 128], F32)
    nc.sync.dma_start(out=e4_sb, in_=emats[64:96, :])
    wt_sb = consts_p.tile([128, 23, H2], BF16)
    nc.sync.dma_start(out=wt_sb, in_=wtail[:, :, :])
    edge_sb = bigres.tile([128, 2], F32, name="edge", tag="edge")
    nc.sync.dma_start(out=edge_sb, in_=edgein[:, :])

    g8_sb = gm_sb[:, 0:16]
    g16_sb = gm_sb[:, 16:24]
    g4_sb = gm_sb[:, 24:56]
    r2d_sb = wt_sb[:, WT_R2D:WT_R2D + 4, :]
    s1w_sb = wt_sb[:, WT_S1:WT_S1 + 9, :]
    e1w_sb = wt_sb[:, WT_E1:WT_E1 + 9, :]
    s2w_sb = wt_sb[:, WT_S2, 0:2]

    epsT = consts_p.tile([32, 1], F32)
    nc.vector.memset(epsT, EPS)
    ident = consts_p.tile([128, 128], F32)
    make_identity(nc, ident)

    def vcol(r):
        return vec_sb[:, r:r + 1]

    # ---- GroupNorm helpers ----
    def stats_from(pstat, src_ap, bias_ap, G, gdim, dst):
        cols = src_ap.free_size()
        assert cols <= 512
        st6 = small.tile([128, 4, 6], F32, name="st6", tag="st6")[:, :1, :]
        nc.vector.bn_stats(out=st6, in_=src_ap)
        stats_tail(pstat, st6, bias_ap, G, gdim, dst)

    def stats_tail(pstat, st6_ap, bias_ap, G, gdim, dst):
        mv = small.tile([128, 2], F32, name="mv", tag="mv")
        nc.vector.bn_aggr(out=mv, in_=st6_ap)
        s12 = small.tile([128, 2], F32, name="s12", tag="s12")
        nc.vector.tensor_scalar_add(s12[:, 0:1], mv[:, 0:1], bias_ap)
        sq = small.tile([128, 1], F32, name="sq", tag="sq")
        nc.vector.tensor_mul(sq, s12[:, 0:1], s12[:, 0:1])
        nc.vector.tensor_add(s12[:, 1:2], mv[:, 1:2], sq)
        pg = pstat.tile([gdim, 2], F32, name="pst", tag="pst")
        nc.tensor.matmul(pg, G[:, :], s12, start=True, stop=True)
        nc.vector.tensor_copy(dst, pg)

    def gn_prep(pstat, E, gdim, stg_view, ni, gG, gB, gC):
        """Batched finalize from group stats (already global): per-channel
        (scale, bias) for ni instances."""
        rm = small.tile([32, 8, 2], F32, name="rmb", tag="rmb")[:gdim, :ni, :]
        sq = small.tile([32, 8], F32, name="sqb", tag="sqb")[:gdim, :ni]
        var = small.tile([32, 8], F32, name="varb", tag="varb")[:gdim, :ni]
        nc.vector.tensor_mul(sq, stg_view[:, :, 0], stg_view[:, :, 0])
        nc.vector.tensor_sub(var, stg_view[:, :, 1], sq)
        nc.scalar.activation(out=var, in_=var, func=AF.Sqrt, bias=epsT[:gdim], scale=1.0)
        nc.vector.reciprocal(rm[:, :, 0], var)
        nc.vector.tensor_copy(rm[:, :, 1], stg_view[:, :, 0])
        pb = pstat.tile([128, 8, 2], F32, name="pstb", tag="pstb")[:, :ni, :]
        nc.tensor.matmul(pb, E[:, :], rm, start=True, stop=True)
        scale = small.tile([128, 8], F32, name="scaleb", tag="scaleb", bufs=2)[:, :ni]
        bias = small.tile([128, 8], F32, name="biasb", tag="biasb", bufs=2)[:, :ni]
        t1 = small.tile([128, 8], F32, name="t1b", tag="t1b")[:, :ni]
        nc.vector.tensor_mul(scale, pb[:, :, 0], vec_sb[:, gG:gG + ni])
        nc.vector.tensor_sub(t1, vec_sb[:, gC:gC + ni], pb[:, :, 1])
        nc.vector.tensor_mul(t1, t1, scale)
        nc.vector.tensor_add(bias, t1, vec_sb[:, gB:gB + ni])
        return scale, bias

    def cc_warm(tag):
        """Dummy tiny AllReduce issued ahead of a real sync point, hidden
        under compute, to keep the collective pipeline warm."""
        if not os.environ.get("KB3_CCWARM"):
            return
        wb = dram.tile([8, 4], F32, name=f"wb_{tag}", tag=f"wb_{tag}")
        wo = dram.tile([8, 4], F32, name=f"wo_{tag}", tag=f"wo_{tag}")
        ws = bigres.tile([8, 4], F32, name=f"ws_{tag}", tag=f"ws_{tag}")
        nc.gpsimd.memset(ws, 1.0)
        nc.gpsimd.dma_start(wb[:, :], ws)
        nc.gpsimd.collective_compute(
            "AllReduce", mybir.AluOpType.add,
            replica_groups=[list(range(NCORES))],
            ins=[wb.opt()], outs=[wo.opt()])
        nc.gpsimd.dma_start(ws, wo[:, :])

    def cc_average(st_tile, flat_parts, flat_cols, tag):
        """AllReduce-add the partial stats tile across the 8 cores, then
        scale by 1/8 (equal per-core element counts -> global mean)."""
        flat = st_tile.rearrange(f"p a b -> p (a b)") if len(st_tile.shape) == 3 else \
            st_tile.rearrange(f"p a b c -> p (a b c)")
        if os.environ.get("KB3_NOCC"):
            # timing-only variant: skip the cross-core exchange (output WRONG)
            nc.vector.tensor_scalar_mul(flat, flat, 1.0)
            return
        if os.environ.get("KB3_CC1ONLY") and tag != "g3":
            nc.vector.tensor_scalar_mul(flat, flat, 1.0)
            return
        ib = dram.tile([flat_parts, flat_cols], F32, name=f"ib_{tag}", tag=f"ib_{tag}")
        ob = dram.tile([flat_parts, flat_cols], F32, name=f"ob_{tag}", tag=f"ob_{tag}")
        nc.gpsimd.dma_start(ib[:, :], flat)
        nc.gpsimd.collective_compute(
            "AllReduce", mybir.AluOpType.add,
            replica_groups=[list(range(NCORES))],
            ins=[ib.opt()], outs=[ob.opt()])
        nc.gpsimd.dma_start(flat, ob[:, :])
        nc.vector.tensor_scalar_mul(flat, flat, 1.0 / NCORES)

    # ---- stage 1: conv1 + GN1 + ReLU + transpose (full T, redundant) ----
    h_sb = [[bigres.tile([128, T], F32, name=f"h{b}{mt}", tag=f"h{b}{mt}") for mt in range(2)] for b in range(B)]
    hT_sb = [[bigres.tile([128, H1], BF16, name=f"ht{b}{tt}", tag=f"ht{b}{tt}") for tt in range(2)] for b in range(B)]
    st1 = bigres.tile([16, B, 2, 2], F32, name="st1", tag="st1")

    with tc.tile_pool(name="ps1", bufs=1, space="PSUM") as ps1:
        ph = {}
        for mt in range(2):
            ph[mt] = ps1.tile([128, B, T], F32, name="ph", tag=f"ph{mt}")
            for idx in range(12):
                j, ct = idx // 4, idx % 4
                nc.tensor.matmul(
                    ph[mt],
                    c1w_sb[:, idx, mt * 128:(mt + 1) * 128],
                    x_sb[:, ct, :, j:j + T],
                    start=(idx == 0), stop=(idx == 11),
                )
            for b in range(B):
                # stage-1 stats are full-T and per-core exact (no collective)
                cols = T
                st6 = small.tile([128, 4, 6], F32, name="st6", tag="st6")[:, :1, :]
                nc.vector.bn_stats(out=st6, in_=ph[mt][:, b, :])
                stats_tail(ps1, st6, vcol(V_C1B + mt), g8_sb, 16, st1[:, b, mt, :])
        sc1, bi1 = gn_prep(ps1, e8_sb, 16,
                           st1.rearrange("g b m s -> g (b m) s"), 4,
                           V_BG1G, V_BG1B, V_BG1C)
        for b in range(B):
            for mt in range(2):
                i = b * 2 + mt
                nc.scalar.activation(out=h_sb[b][mt], in_=ph[mt][:, b, :],
                                     func=AF.Relu, bias=sc_bias_col(bi1, i), scale=sc_bias_col(sc1, i))
            for tt in range(2):
                for mt in range(2):
                    pt = ps1.tile([128, 128], F32, name="pt", tag="pt", bufs=2)
                    nc.tensor.transpose(pt, h_sb[b][mt][:, tt * 128:(tt + 1) * 128], ident)
                    nc.vector.tensor_copy(hT_sb[b][tt][:, mt * 128:(mt + 1) * 128], pt)

    # ---- stage 2': sampling GEMM on the core-local window ----
    ssamp = [bigres.tile([128, N, B, OC], BF16, name=f"ss{ct}", tag=f"ss{ct}") for ct in range(2)]
    eng_rot = [0]

    def rot_copy(dst, src):
        # PSUM -> SBUF: only Act (scalar) and DVE (vector) can read PSUM
        e = eng_rot[0] % 2
        eng_rot[0] += 1
        if e == 0:
            nc.scalar.activation(out=dst, in_=src, func=AF.Copy)
        else:
            nc.vector.tensor_copy(dst, src)

    # stage 2+3 interleaved: sampling for sample n feeds the Conv3d
    # accumulation for k=2n,2n+1 while sampling for n+1 proceeds.
    # py holds only the owned 256 cols (1 PSUM bank per ot); the 16 halo
    # cols accumulate in a separate tiny pass afterwards.
    y_sb = [bigres.tile([128, B, OC], BF16, name=f"y{ot}", tag=f"y{ot}") for ot in range(4)]
    st3 = bigres.tile([8, B, 4, 2], F32, name="st3", tag="st3")
    st2 = bigres.tile([32, B, 2], F32, name="st2", tag="st2")
    sth = bigres.tile([32, B, 2, 2], F32, name="sth", tag="sth")

    # stage 3 runs in two ot-waves of half-width (136-col) PSUM tiles so the
    # full 272-col window accumulates in-bank; wave 0 (ot 0,1) interleaves
    # with the sampling loop, wave 1 (ot 2,3) streams right after.
    HC = OC // 2  # 136

    def wave_matmuls(ot_pair, k, pyt):
        n, ct = k >> 1, k & 1
        for oi, ot in enumerate(ot_pair):
            for hh in range(2):
                nc.tensor.matmul(
                    pyt[oi * 2 + hh],
                    r3d_sb[:, k, ot * 128:(ot + 1) * 128],
                    ssamp[ct][:, n, :, hh * HC:(hh + 1) * HC],
                    start=(k == 0), stop=(k == 63),
                )

    with tc.tile_pool(name="ps23", bufs=1, space="PSUM") as ps23:
        pyw = [ps23.tile([128, B, HC], F32, name="pyw", tag=f"pyw{i}") for i in range(4)]
        for n in range(N):
            for ct in range(2):
                for b in range(B):
                    ps2 = ps23.tile([128, OC], F32, name="ps2", tag="ps2", bufs=3)
                    for tt in range(2):
                        nc.tensor.matmul(
                            ps2,
                            hT_sb[b][tt][:, ct * 128:(ct + 1) * 128],
                            mp_sb[:, tt, n, :],
                            start=(tt == 0), stop=(tt == 1),
                        )
                    rot_copy(ssamp[ct][:, n, b, :], ps2)
            for ct in range(2):
                wave_matmuls((0, 1), n * 2 + ct, pyw)
        for i, (ot, hh) in enumerate([(o, h) for o in (0, 1) for h in range(2)]):
            for b in range(B):
                rot_copy(y_sb[ot][:, b, hh * HC:(hh + 1) * HC], pyw[i][:, b, :])

    cc_warm("w1")
    with tc.tile_pool(name="ps3b", bufs=1, space="PSUM") as ps3b:
        pyw2 = [ps3b.tile([128, B, HC], F32, name="pyw2", tag=f"pyw2{i}") for i in range(4)]
        for k in range(64):
            wave_matmuls((2, 3), k, pyw2)
        for i, (ot, hh) in enumerate([(o, h) for o in (2, 3) for h in range(2)]):
            for b in range(B):
                rot_copy(y_sb[ot][:, b, hh * HC:(hh + 1) * HC], pyw2[i][:, b, :])

    fpre = bigres.tile([128, B, OC], BF16, name="fpre", tag="fpre")
    f_sb = [bigres.tile([128, TL + 2, W + 2], BF16, name=f"f{b}", tag=f"f{b}") for b in range(B)]
    hpre = [bigres.tile([128, TC * W], BF16, name=f"hpre{i}", tag=f"hpre{i}") for i in range(4)]

    with tc.tile_pool(name="ps45", bufs=1, space="PSUM") as ps45:
        for ot in range(4):
            for b in range(B):
                stats_from(ps45, y_sb[ot][:, b, OWN_LO:OWN_HI], vcol(V_R3DB + ot),
                           g16_sb, 8, st3[:, b, ot, :])

        # ---- collective 1 + GN3 prep ----
        cc_average(st3, 8, 16, "g3")
        cc_warm("w2")
        sc3, bi3 = gn_prep(ps45, e16_sb, 8, st3.rearrange("g b o s -> g (b o) s"), 8,
                           V_BG3G, V_BG3B, V_BG3C)
        for b in range(B):
            for ot in range(4):
                i = b * 4 + ot
                nc.scalar.activation(out=y_sb[ot][:, b, :], in_=y_sb[ot][:, b, :],
                                     func=AF.Relu, bias=bi3[:, i:i + 1], scale=sc3[:, i:i + 1])
            pf = ps45.tile([128, OC], F32, name="pf", tag="pf", bufs=2)
            for ot in range(4):
                nc.tensor.matmul(pf, r2d_sb[:, ot, :], y_sb[ot][:, b, :],
                                 start=(ot == 0), stop=(ot == 3))
            rot_copy(fpre[:, b, :], pf)
            stats_from(ps45, fpre[:, b, OWN_LO:OWN_HI], vcol(V_R2DB), g4_sb, 32, st2[:, b, :])
            nc.vector.memset(f_sb[b], 0.0)

        # ---- collective 2 + GN2 prep/apply ----
        cc_average(st2, 32, 2 * B, "g2")
        cc_warm("w3")
        sc2, bi2 = gn_prep(ps45, e4_sb, 32, st2, 2, V_BG2G, V_BG2B, V_BG2C)
        for b in range(B):
            nc.scalar.activation(out=f_sb[b][:, 1:TL + 1, 1:W + 1], in_=fpre[:, b, :],
                                 func=AF.Relu, bias=bi2[:, b:b + 1], scale=sc2[:, b:b + 1])
            # conv zero-padding at the global t boundary: the halo columns are
            # real data on interior cores, zero on edge cores (edge in {0,1}).
            for side in range(2):
                scE = small.tile([128, 1], F32, name="scE", tag="scE", bufs=2)
                biE = small.tile([128, 1], F32, name="biE", tag="biE", bufs=2)
                nc.vector.tensor_mul(scE, sc2[:, b:b + 1], edge_sb[:, side:side + 1])
                nc.vector.tensor_mul(biE, bi2[:, b:b + 1], edge_sb[:, side:side + 1])
                lt = 0 if side == 0 else TL - 1
                nc.scalar.activation(out=f_sb[b][:, lt + 1:lt + 2, 1:W + 1],
                                     in_=fpre[:, b, lt * W:(lt + 1) * W],
                                     func=AF.Relu, bias=biE, scale=scE)

        # ---- stage 5: heads ----
        for b in range(B):
            for hd in range(2):
                w_sb = s1w_sb if hd == 0 else e1w_sb
                i = b * 2 + hd
                phd = ps45.tile([128, TC * W], F32, name="phd", tag="phd", bufs=2)
                for tap in range(9):
                    kt, kw = tap // 3, tap % 3
                    # out t' in [0,32) maps to f_sb col (t'+1+kt, kw) in the
                    # (TL+2, W+2) padded layout
                    nc.tensor.matmul(phd, w_sb[:, tap, :],
                                     f_sb[b][:, 1 + kt:1 + kt + TC, kw:kw + W],
                                     start=(tap == 0), stop=(tap == 8))
                rot_copy(hpre[i], phd)
                stats_from(ps45, hpre[i], vcol(V_S1B + hd), g4_sb, 32, sth[:, b, hd, :])

        # ---- collective 3 + head GN + final 1x1 + sigmoid ----
        cc_average(sth, 32, 2 * B * 2, "gh")
        sch, bih = gn_prep(ps45, e4_sb, 32, sth.rearrange("g b h s -> g (b h) s"), 4,
                           V_BHG, V_BHB, V_BHC)
        for b in range(B):
            for hd in range(2):
                i = b * 2 + hd
                nc.scalar.activation(out=hpre[i], in_=hpre[i], func=AF.Relu,
                                     bias=bih[:, i:i + 1], scale=sch[:, i:i + 1])
                po = ps45.tile([1, TC * W], F32, name="po", tag="po", bufs=2)
                nc.tensor.matmul(po, s2w_sb[:, hd:hd + 1], hpre[i], start=True, stop=True)
                o_one = bigres.tile([1, TC * W], F32, name="o_one", tag="o_one", bufs=2)
                nc.scalar.activation(out=o_one, in_=po, func=AF.Sigmoid,
                                     bias=vec_sb[0:1, V_S2B + hd:V_S2B + hd + 1], scale=1.0)
                nc.sync.dma_start(out=outd[b, hd, :, :], in_=o_one)


# stage-1 batched GN prep needs bg1 vec rows; extend the table
V_BG1G = NVEC
V_BG1B = NVEC + 4
V_BG1C = NVEC + 8
NVEC_TOTAL = NVEC + 12


def sc_bias_col(mat, i):
    return mat[:, i:i + 1]


def _f32(a):
    return np.ascontiguousarray(np.asarray(a, dtype=np.float32))


def _prep_consts(inputs):
    c1_w = _f32(inputs["c1_w"])
    r3d_w = _f32(inputs["r3d_w"])[:, :, :, 0, 0]
    r2d_w = _f32(inputs["r2d_w"])[:, :, 0, 0]
    s1_w = _f32(inputs["s1_w"])
    e1_w = _f32(inputs["e1_w"])
    s2_w = _f32(inputs["s2_w"])[0, :, 0, 0]
    e2_w = _f32(inputs["e2_w"])[0, :, 0, 0]

    # conv1 weights: [c, j*4+ct, m] = c1_w[m, ct*128+c, j]
    a = c1_w.transpose(1, 2, 0).reshape(4, 128, 3, H1)
    c1w_h = np.ascontiguousarray(a.transpose(1, 2, 0, 3).reshape(128, 12, H1)).astype(BFNP)

    # r3d weights: [c, n*2+ct, o] = r3d_w[o, ct*128+c, n]
    a = r3d_w.transpose(1, 2, 0).reshape(2, 128, N, H3)
    r3d_h = np.ascontiguousarray(a.transpose(1, 2, 0, 3).reshape(128, 64, H3)).astype(BFNP)

    wtail = np.zeros((128, 23, H2), np.float32)
    wtail[:, WT_R2D:WT_R2D + 4, :] = r2d_w.T.reshape(4, 128, H2).transpose(1, 0, 2)
    wtail[:, WT_S1:WT_S1 + 9, :] = s1_w.transpose(1, 2, 3, 0).reshape(128, 9, H2)
    wtail[:, WT_E1:WT_E1 + 9, :] = e1_w.transpose(1, 2, 3, 0).reshape(128, 9, H2)
    wtail[:, WT_S2, 0] = s2_w
    wtail[:, WT_S2, 1] = e2_w
    wtail_h = wtail.astype(BFNP)

    ch = np.arange(128)
    g8 = (ch[:, None] // 8 == np.arange(16)[None, :]).astype(np.float32)
    g16 = (ch[:, None] // 16 == np.arange(8)[None, :]).astype(np.float32)
    g4 = (ch[:, None] // 4 == np.arange(32)[None, :]).astype(np.float32)
    gmats = np.concatenate([g8 / 8.0, g16 / 16.0, g4 / 4.0], axis=1)
    emats = np.zeros((96, 128), np.float32)
    emats[0:16] = g8.T
    emats[32:40] = g16.T
    emats[64:96] = g4.T

    vecs = np.zeros((NVEC_TOTAL, 128), np.float32)
    vecs[V_C1B:V_C1B + 2] = _f32(inputs["c1_b"]).reshape(2, 128)
    vecs[V_GN1G:V_GN1G + 2] = _f32(inputs["gn1_g"]).reshape(2, 128)
    vecs[V_GN1B:V_GN1B + 2] = _f32(inputs["gn1_b"]).reshape(2, 128)
    vecs[V_R3DB:V_R3DB + 4] = _f32(inputs["r3d_b"]).reshape(4, 128)
    vecs[V_GN3G:V_GN3G + 4] = _f32(inputs["gn3_g"]).reshape(4, 128)
    vecs[V_GN3B:V_GN3B + 4] = _f32(inputs["gn3_b"]).reshape(4, 128)
    vecs[V_R2DB] = _f32(inputs["r2d_b"])
    vecs[V_GN2G] = _f32(inputs["gn2_g"])
    vecs[V_GN2B] = _f32(inputs["gn2_b"])
    vecs[V_S1B] = _f32(inputs["s1_b"])
    vecs[V_E1B] = _f32(inputs["e1_b"])
    vecs[V_SGNG] = _f32(inputs["sgn_g"])
    vecs[V_SGNB] = _f32(inputs["sgn_b"])
    vecs[V_EGNG] = _f32(inputs["egn_g"])
    vecs[V_EGNB] = _f32(inputs["egn_b"])
    vecs[V_S2B] = _f32(inputs["s2_b"])[0]
    vecs[V_E2B] = _f32(inputs["e2_b"])[0]
    gn3g4 = _f32(inputs["gn3_g"]).reshape(4, 128)
    gn3b4 = _f32(inputs["gn3_b"]).reshape(4, 128)
    r3db4 = _f32(inputs["r3d_b"]).reshape(4, 128)
    for i, (b, ot) in enumerate([(b, ot) for b in range(B) for ot in range(4)]):
        vecs[V_BG3G + i] = gn3g4[ot]
        vecs[V_BG3B + i] = gn3b4[ot]
        vecs[V_BG3C + i] = r3db4[ot]
    for b in range(B):
        vecs[V_BG2G + b] = _f32(inputs["gn2_g"])
        vecs[V_BG2B + b] = _f32(inputs["gn2_b"])
        vecs[V_BG2C + b] = _f32(inputs["r2d_b"])
    hg = [_f32(inputs["sgn_g"]), _f32(inputs["egn_g"])]
    hb = [_f32(inputs["sgn_b"]), _f32(inputs["egn_b"])]
    hc = [_f32(inputs["s1_b"]), _f32(inputs["e1_b"])]
    for i, (b, hd) in enumerate([(b, hd) for b in range(B) for hd in range(2)]):
        vecs[V_BHG + i] = hg[hd]
        vecs[V_BHB + i] = hb[hd]
        vecs[V_BHC + i] = hc[hd]
    gn1g2 = _f32(inputs["gn1_g"]).reshape(2, 128)
    gn1b2 = _f32(inputs["gn1_b"]).reshape(2, 128)
    c1b2 = _f32(inputs["c1_b"]).reshape(2, 128)
    for i, (b, mt) in enumerate([(b, mt) for b in range(B) for mt in range(2)]):
        vecs[V_BG1G + i] = gn1g2[mt]
        vecs[V_BG1B + i] = gn1b2[mt]
        vecs[V_BG1C + i] = c1b2[mt]

    return {
        "c1w": c1w_h, "r3dw": r3d_h, "wtail": wtail_h,
        "gmats": gmats, "emats": emats, "vecs": vecs,
        "mask": _f32(inputs["sample_mask"]).reshape(T, N, T, W),
    }


def _fingerprint(inputs):
    h = hashlib.sha1()
    for k in sorted(inputs.keys()):
        if k == "x":
            continue
        a = np.asarray(inputs[k])
        h.update(k.encode())
        h.update(str(a.shape).encode())
        h.update(str(a.dtype).encode())
        flat = a.reshape(-1)
        step = max(1, flat.size // 65536)
        h.update(np.ascontiguousarray(flat[::step]).tobytes())
    return h.hexdigest()


_module_cache = {}


def _get_module(inputs=None):
    if inputs is None:
        if "nc" not in _module_cache:
            raise RuntimeError("module not built yet; call kernel() first")
        return _module_cache["nc"]
    fp = _fingerprint(inputs)
    if _module_cache.get("fp") != fp:
        consts = _prep_consts(inputs)
        _module_cache["nc"] = _build(consts)
        _module_cache["fp"] = fp
        _module_cache["consts"] = consts
    return _module_cache["nc"]


def _prep(inputs):
    """Per-core input maps: full x, per-core mask slice + edge mask."""
    x_h = np.ascontiguousarray(_f32(inputs["x"]).astype(BFNP))
    mask = _f32(inputs["sample_mask"]).reshape(T, N, T, W)
    maps = []
    for c in range(NCORES):
        mp = np.zeros((2, 128, N, OC), np.float32)
        for lt in range(TL):
            gt = TC * c - 1 + lt
            if 0 <= gt < T:
                # mp[tt, p, n, lt*W+w] = mask[tt*128+p, n, gt, w]
                mslice = mask[:, :, gt, :]  # (T', N, W)
                mp[:, :, :, lt * W:(lt + 1) * W] = mslice.reshape(2, 128, N, W)
        edge = np.ones((128, 2), np.float32)
        if c == 0:
            edge[:, 0] = 0.0
        if c == NCORES - 1:
            edge[:, 1] = 0.0
        maps.append({
            "x_in": x_h,
            "maskp": np.ascontiguousarray(mp.astype(BFNP)),
            "edge": edge,
        })
    return maps


def kernel(**inputs) -> np.ndarray:
    nc = _get_module(inputs)
    in_maps = _prep(inputs)
    from concourse.bass_utils import run_bass_kernel_spmd
    res = run_bass_kernel_spmd(nc, in_maps, list(range(NCORES)))
    full = np.zeros((B, 2, T, W), np.float32)
    for c in range(NCORES):
        full[:, :, TC * c:TC * (c + 1), :] = res.results[c]["out"].astype(np.float32)
    return np.ascontiguousarray(full)


# revision 23
# speedup vs baseline: 1.0608x; 1.0369x over previous
"""BEM (boundary evaluation module) Trainium2 kernel, v3: t-sharded SPMD.

Each of the 8 cores owns a 32-wide slice of the T=256 axis.  Stage 1
(conv1+GN1, ~2% of the FLOPs) is computed redundantly on every core; the
expensive sampling GEMM and Conv3d reduction (stages 2-3) and the whole
tail run only on each core's 34-wide haloed window (+-1 t for the heads'
3x3 convs).  GroupNorm statistics are global over (T, W), so the three GN
points after stage 3 exchange per-core partial moments with tiny (<=1KB)
AllReduces (~1-2us each measured).  Everything core-dependent arrives as
per-core ExternalInput data (the mask slice in local-t coordinates, an
edge mask for the conv zero-padding, and a per-core output slice), so all
cores share one SPMD instruction stream.

Weights are baked into the NEFF as inline consts; the per-core mask slice
(4.5MB vs the full 64MB) is a per-core input.
"""

import hashlib
import os
import sys

import numpy as np

for _p in ("/opt/trn_rl_repo", "/root/.axon_site/_ro/trn_rl_repo"):
    if _p not in sys.path:
        sys.path.append(_p)

import ml_dtypes  # noqa: E402
import concourse.bass as bass  # noqa: E402
import concourse.bacc as bacc  # noqa: E402
import concourse.tile as tile  # noqa: E402
import concourse.mybir as mybir  # noqa: E402
from contextlib import ExitStack  # noqa: E402
from concourse.masks import make_identity  # noqa: E402

F32 = mybir.dt.float32
BF16 = mybir.dt.bfloat16
AF = mybir.ActivationFunctionType
BFNP = ml_dtypes.bfloat16

B = 2
DIM = 512
T = 256
H1 = 256
H3 = 512
H2 = 128
N = 32
W = 8
NCORES = 8
EPS = 1e-5
TC = 32            # owned t's per core
TL = TC + 2        # local window incl halo
OC = TL * W        # 272 sampling cols per (n, b)
OWN_LO, OWN_HI = W, W + TC * W   # owned col range [8, 264) within OC

# rows of the packed per-channel vector table (same layout as v2)
V_C1B = 0
V_GN1G = 2
V_GN1B = 4
V_R3DB = 6
V_GN3G = 10
V_GN3B = 14
V_R2DB = 18
V_GN2G = 19
V_GN2B = 20
V_S1B = 21
V_E1B = 22
V_SGNG = 23
V_SGNB = 24
V_EGNG = 25
V_EGNB = 26
V_S2B = 27
V_E2B = 28
V_BG3G = 29
V_BG3B = 37
V_BG3C = 45
V_BG2G = 53
V_BG2B = 55
V_BG2C = 57
V_BHG = 59
V_BHB = 63
V_BHC = 67
NVEC = 71

WT_R2D = 0
WT_S1 = 4
WT_E1 = 13
WT_S2 = 22


def _build(consts, reps=1):
    nc = bacc.Bacc("TRN2", target_bir_lowering=False, debug=False, num_devices=NCORES)

    xin = nc.declare_dram_parameter("x_in", [B, DIM, T], BF16, isOutput=False)
    mpin = nc.declare_dram_parameter("maskp", [2, 128, N, OC], BF16, isOutput=False)
    edgein = nc.declare_dram_parameter("edge", [128, 2], F32, isOutput=False)
    outd = nc.declare_dram_parameter("out", [B, 2, TC, W], F32, isOutput=True)

    c1w = nc.inline_tensor(consts["c1w"], name="c1w")
    r3dw = nc.inline_tensor(consts["r3dw"], name="r3dw")
    wtail = nc.inline_tensor(consts["wtail"], name="wtail")
    gmats = nc.inline_tensor(consts["gmats"], name="gmats")
    emats = nc.inline_tensor(consts["emats"], name="emats")
    vecsd = nc.inline_tensor(consts["vecs"], name="vecs")

    with tile.TileContext(nc) as tc, ExitStack() as ctx:
        consts_p = ctx.enter_context(tc.tile_pool(name="consts", bufs=1))
        bigres = ctx.enter_context(tc.tile_pool(name="bigres", bufs=1))
        small = ctx.enter_context(tc.tile_pool(name="small", bufs=8))
        dram = ctx.enter_context(tc.tile_pool(name="dram", bufs=1, space="DRAM"))

        for _rep in range(reps):
            _body(nc, tc, consts_p, bigres, small, dram,
                  xin, mpin, edgein, outd, c1w, r3dw, wtail, gmats, emats, vecsd)

    nc.compile()
    return nc


def _body(nc, tc, consts_p, bigres, small, dram,
          xin, mpin, edgein, outd, c1w, r3dw, wtail, gmats, emats, vecsd):
    # ---- loads ----
    r3d_sb = bigres.tile([128, 64, H3], BF16, name="r3d", tag="r3d")
    for _ci in range(8):
        nc.scalar.dma_start(out=r3d_sb[:, _ci * 4:(_ci + 1) * 4, :],
                            in_=r3dw[:, _ci * 4:(_ci + 1) * 4, :])
    for _ci in range(8, 16):
        nc.gpsimd.dma_start(out=r3d_sb[:, _ci * 4:(_ci + 1) * 4, :],
                            in_=r3dw[:, _ci * 4:(_ci + 1) * 4, :])

    mp_sb = bigres.tile([128, 2, N, OC], BF16, name="mp", tag="mp")
    for tt in range(2):
        for nh in range(2):
            nc.sync.dma_start(out=mp_sb[:, tt, nh * 16:(nh + 1) * 16, :],
                              in_=mpin[tt, :, nh * 16:(nh + 1) * 16, :])

    x_sb = bigres.tile([128, 4, B, T + 2], BF16, name="xsb", tag="xsb")
    nc.vector.memset(x_sb[:, :, :, 0:1], 0.0)
    nc.vector.memset(x_sb[:, :, :, T + 1:T + 2], 0.0)
    for b in range(B):
        nc.sync.dma_start(
            out=x_sb[:, :, b, 1:T + 1],
            in_=bass.AP(tensor=xin, offset=b * DIM * T,
                        ap=[[T, 128], [128 * T, 4], [1, T]]))
    c1w_sb = consts_p.tile([128, 12, H1], BF16)
    nc.sync.dma_start(out=c1w_sb, in_=c1w[:, :, :])
    vec_sb = consts_p.tile([128, NVEC_TOTAL], F32)
    nc.sync.dma_start(out=vec_sb, in_=bass.AP(tensor=vecsd, offset=0, ap=[[1, 128], [128, NVEC_TOTAL]]))
    gm_sb = consts_p.tile([128, 56], F32)
    nc.sync.dma_start(out=gm_sb, in_=gmats[:, :])
    e8_sb = consts_p.tile([16, 128], F32)
    nc.sync.dma_start(out=e8_sb, in_=emats[0:16, :])
    e16_sb = consts_p.tile([8, 128], F32)
    nc.sync.dma_start(out=e16_sb, in_=emats[32:40, :])
    e4_sb = consts_p.tile([32, 128], F32)
    nc.sync.dma_start(out=e4_sb, in_=emats[64:96, :])
    wt_sb = consts_p.tile([128, 23, H2], BF16)
    nc.sync.dma_start(out=wt_sb, in_=wtail[:, :, :])
    edge_sb = bigres.tile([128, 2], F32, name="edge", tag="edge")
    nc.sync.dma_start(out=edge_sb, in_=edgein[:, :])

    g8_sb = gm_sb[:, 0:16]
    g16_sb = gm_sb[:, 16:24]
    g4_sb = gm_sb[:, 24:56]
    r2d_sb = wt_sb[:, WT_R2D:WT_R2D + 4, :]
    s1w_sb = wt_sb[:, WT_S1:WT_S1 + 9, :]
    e1w_sb = wt_sb[:, WT_E1:WT_E1 + 9, :]
    s2w_sb = wt_sb[:, WT_S2, 0:2]

    epsT = consts_p.tile([32, 1], F32)
    nc.vector.memset(epsT, EPS)
    ident = consts_p.tile([128, 128], F32)
    make_identity(nc, ident)

    def vcol(r):
        return vec_sb[:, r:r + 1]

    # ---- GroupNorm helpers ----
    def stats_from(pstat, src_ap, bias_ap, G, gdim, dst):
        cols = src_ap.free_size()
        assert cols <= 512
        st6 = small.tile([128, 4, 6], F32, name="st6", tag="st6")[:, :1, :]
        nc.vector.bn_stats(out=st6, in_=src_ap)
        stats_tail(pstat, st6, bias_ap, G, gdim, dst)

    def stats_tail(pstat, st6_ap, bias_ap, G, gdim, dst):
        mv = small.tile([128, 2], F32, name="mv", tag="mv")
        nc.vector.bn_aggr(out=mv, in_=st6_ap)
        s12 = small.tile([128, 2], F32, name="s12", tag="s12")
        nc.vector.tensor_scalar_add(s12[:, 0:1], mv[:, 0:1], bias_ap)
        sq = small.tile([128, 1], F32, name="sq", tag="sq")
        nc.vector.tensor_mul(sq, s12[:, 0:1], s12[:, 0:1])
        nc.vector.tensor_add(s12[:, 1:2], mv[:, 1:2], sq)
        pg = pstat.tile([gdim, 2], F32, name="pst", tag="pst")
        nc.tensor.matmul(pg, G[:, :], s12, start=True, stop=True)
        nc.vector.tensor_copy(dst, pg)

    def gn_prep(pstat, E, gdim, stg_view, ni, gG, gB, gC):
        """Batched finalize from group stats (already global): per-channel
        (scale, bias) for ni instances."""
        rm = small.tile([32, 8, 2], F32, name="rmb", tag="rmb")[:gdim, :ni, :]
        sq = small.tile([32, 8], F32, name="sqb", tag="sqb")[:gdim, :ni]
        var = small.tile([32, 8], F32, name="varb", tag="varb")[:gdim, :ni]
        nc.vector.tensor_mul(sq, stg_view[:, :, 0], stg_view[:, :, 0])
        nc.vector.tensor_sub(var, stg_view[:, :, 1], sq)
        nc.scalar.activation(out=var, in_=var, func=AF.Sqrt, bias=epsT[:gdim], scale=1.0)
        nc.vector.reciprocal(rm[:, :, 0], var)
        nc.vector.tensor_copy(rm[:, :, 1], stg_view[:, :, 0])
        pb = pstat.tile([128, 8, 2], F32, name="pstb", tag="pstb")[:, :ni, :]
        nc.tensor.matmul(pb, E[:, :], rm, start=True, stop=True)
        scale = small.tile([128, 8], F32, name="scaleb", tag="scaleb", bufs=2)[:, :ni]
        bias = small.tile([128, 8], F32, name="biasb", tag="biasb", bufs=2)[:, :ni]
        t1 = small.tile([128, 8], F32, name="t1b", tag="t1b")[:, :ni]
        nc.vector.tensor_mul(scale, pb[:, :, 0], vec_sb[:, gG:gG + ni])
        nc.vector.tensor_sub(t1, vec_sb[:, gC:gC + ni], pb[:, :, 1])
        nc.vector.tensor_mul(t1, t1, scale)
        nc.vector.tensor_add(bias, t1, vec_sb[:, gB:gB + ni])
        return scale, bias

    def cc_warm(tag):
        """Dummy tiny AllReduce issued ahead of a real sync point, hidden
        under compute, to keep the collective pipeline warm."""
        if not os.environ.get("KB3_CCWARM"):
            return
        wb = dram.tile([8, 4], F32, name=f"wb_{tag}", tag=f"wb_{tag}")
        wo = dram.tile([8, 4], F32, name=f"wo_{tag}", tag=f"wo_{tag}")
        ws = bigres.tile([8, 4], F32, name=f"ws_{tag}", tag=f"ws_{tag}")
        nc.gpsimd.memset(ws, 1.0)
        nc.gpsimd.dma_start(wb[:, :], ws)
        nc.gpsimd.collective_compute(
            "AllReduce", mybir.AluOpType.add,
            replica_groups=[list(range(NCORES))],
            ins=[wb.opt()], outs=[wo.opt()])
        nc.gpsimd.dma_start(ws, wo[:, :])

    def cc_average(st_tile, flat_parts, flat_cols, tag):
        """AllReduce-add the partial stats tile across the 8 cores, then
        scale by 1/8 (equal per-core element counts -> global mean)."""
        flat = st_tile.rearrange(f"p a b -> p (a b)") if len(st_tile.shape) == 3 else \
            st_tile.rearrange(f"p a b c -> p (a b c)")
        if os.environ.get("KB3_NOCC"):
            # timing-only variant: skip the cross-core exchange (output WRONG)
            nc.vector.tensor_scalar_mul(flat, flat, 1.0)
            return
        if os.environ.get("KB3_CC1ONLY") and tag != "g3":
            nc.vector.tensor_scalar_mul(flat, flat, 1.0)
            return
        ib = dram.tile([flat_parts, flat_cols], F32, name=f"ib_{tag}", tag=f"ib_{tag}")
        ob = dram.tile([flat_parts, flat_cols], F32, name=f"ob_{tag}", tag=f"ob_{tag}")
        nc.gpsimd.dma_start(ib[:, :], flat)
        nc.gpsimd.collective_compute(
            "AllReduce", mybir.AluOpType.add,
            replica_groups=[list(range(NCORES))],
            ins=[ib.opt()], outs=[ob.opt()])
        nc.gpsimd.dma_start(flat, ob[:, :])
        nc.vector.tensor_scalar_mul(flat, flat, 1.0 / NCORES)

    # ---- stage 1: conv1 + GN1 + ReLU + transpose (full T, redundant) ----
    h_sb = [[bigres.tile([128, T], F32, name=f"h{b}{mt}", tag=f"h{b}{mt}") for mt in range(2)] for b in range(B)]
    hT_sb = [[bigres.tile([128, H1], BF16, name=f"ht{b}{tt}", tag=f"ht{b}{tt}") for tt in range(2)] for b in range(B)]
    st1 = bigres.tile([16, B, 2, 2], F32, name="st1", tag="st1")

    with tc.tile_pool(name="ps1", bufs=1, space="PSUM") as ps1:
        ph = {}
        for mt in range(2):
            ph[mt] = ps1.tile([128, B, T], F32, name="ph", tag=f"ph{mt}")
            for idx in range(12):
                j, ct = idx // 4, idx % 4
                nc.tensor.matmul(
                    ph[mt],
                    c1w_sb[:, idx, mt * 128:(mt + 1) * 128],
                    x_sb[:, ct, :, j:j + T],
                    start=(idx == 0), stop=(idx == 11),
                )
            for b in range(B):
                # stage-1 stats are full-T and per-core exact (no collective)
                cols = T
                st6 = small.tile([128, 4, 6], F32, name="st6", tag="st6")[:, :1, :]
                nc.vector.bn_stats(out=st6, in_=ph[mt][:, b, :])
                stats_tail(ps1, st6, vcol(V_C1B + mt), g8_sb, 16, st1[:, b, mt, :])
        sc1, bi1 = gn_prep(ps1, e8_sb, 16,
                           st1.rearrange("g b m s -> g (b m) s"), 4,
                           V_BG1G, V_BG1B, V_BG1C)
        for b in range(B):
            for mt in range(2):
                i = b * 2 + mt
                nc.scalar.activation(out=h_sb[b][mt], in_=ph[mt][:, b, :],
                                     func=AF.Relu, bias=sc_bias_col(bi1, i), scale=sc_bias_col(sc1, i))
            for tt in range(2):
                for mt in range(2):
                    pt = ps1.tile([128, 128], F32, name="pt", tag="pt", bufs=2)
                    nc.tensor.transpose(pt, h_sb[b][mt][:, tt * 128:(tt + 1) * 128], ident)
                    nc.vector.tensor_copy(hT_sb[b][tt][:, mt * 128:(mt + 1) * 128], pt)

    # ---- stage 2': sampling GEMM on the core-local window ----
    ssamp = [bigres.tile([128, N, B, OC], BF16, name=f"ss{ct}", tag=f"ss{ct}") for ct in range(2)]
    eng_rot = [0]

    def rot_copy(dst, src):
        # PSUM -> SBUF: only Act (scalar) and DVE (vector) can read PSUM
        e = eng_rot[0] % 2
        eng_rot[0] += 1
        if e == 0:
            nc.scalar.activation(out=dst, in_=src, func=AF.Copy)
        else:
            nc.vector.tensor_copy(dst, src)

    # stage 2+3 interleaved: sampling for sample n feeds the Conv3d
    # accumulation for k=2n,2n+1 while sampling for n+1 proceeds.
    # py holds only the owned 256 cols (1 PSUM bank per ot); the 16 halo
    # cols accumulate in a separate tiny pass afterwards.
    y_sb = [bigres.tile([128, B, OC], BF16, name=f"y{ot}", tag=f"y{ot}") for ot in range(4)]
    st3 = bigres.tile([8, B, 4, 2], F32, name="st3", tag="st3")
    st2 = bigres.tile([32, B, 2], F32, name="st2", tag="st2")
    sth = bigres.tile([32, B, 2, 2], F32, name="sth", tag="sth")

    # stage 3 runs in two ot-waves of half-width (136-col) PSUM tiles so the
    # full 272-col window accumulates in-bank; wave 0 (ot 0,1) interleaves
    # with the sampling loop, wave 1 (ot 2,3) streams right after.
    HC = OC // 2  # 136

    def wave_matmuls(ot_pair, k, pyt):
        n, ct = k >> 1, k & 1
        for oi, ot in enumerate(ot_pair):
            for hh in range(2):
                nc.tensor.matmul(
                    pyt[oi * 2 + hh],
                    r3d_sb[:, k, ot * 128:(ot + 1) * 128],
                    ssamp[ct][:, n, :, hh * HC:(hh + 1) * HC],
                    start=(k == 0), stop=(k == 63),
                )

    with tc.tile_pool(name="ps23", bufs=1, space="PSUM") as ps23:
        pyw = [ps23.tile([128, B, HC], F32, name="pyw", tag=f"pyw{i}") for i in range(4)]
        for n in range(N):
            for ct in range(2):
                for b in range(B):
                    ps2 = ps23.tile([128, OC], F32, name="ps2", tag="ps2", bufs=3)
                    for tt in range(2):
                        nc.tensor.matmul(
                            ps2,
                            hT_sb[b][tt][:, ct * 128:(ct + 1) * 128],
                            mp_sb[:, tt, n, :],
                            start=(tt == 0), stop=(tt == 1),
                        )
                    rot_copy(ssamp[ct][:, n, b, :], ps2)
            for ct in range(2):
                wave_matmuls((0, 1), n * 2 + ct, pyw)
        for i, (ot, hh) in enumerate([(o, h) for o in (0, 1) for h in range(2)]):
            for b in range(B):
                rot_copy(y_sb[ot][:, b, hh * HC:(hh + 1) * HC], pyw[i][:, b, :])

    cc_warm("w1")
    with tc.tile_pool(name="ps3b", bufs=1, space="PSUM") as ps3b:
        pyw2 = [ps3b.tile([128, B, HC], F32, name="pyw2", tag=f"pyw2{i}") for i in range(4)]
        for k in range(64):
            wave_matmuls((2, 3), k, pyw2)
        for i, (ot, hh) in enumerate([(o, h) for o in (2, 3) for h in range(2)]):
            for b in range(B):
                rot_copy(y_sb[ot][:, b, hh * HC:(hh + 1) * HC], pyw2[i][:, b, :])

    fpre = bigres.tile([128, B, OC], BF16, name="fpre", tag="fpre")
    f_sb = [bigres.tile([128, TL + 2, W + 2], BF16, name=f"f{b}", tag=f"f{b}") for b in range(B)]
    hpre = [bigres.tile([128, TC * W], BF16, name=f"hpre{i}", tag=f"hpre{i}") for i in range(4)]

    with tc.tile_pool(name="ps45", bufs=1, space="PSUM") as ps45:
        for ot in range(4):
            for b in range(B):
                stats_from(ps45, y_sb[ot][:, b, OWN_LO:OWN_HI], vcol(V_R3DB + ot),
                           g16_sb, 8, st3[:, b, ot, :])

        # ---- collective 1 + GN3 prep ----
        cc_average(st3, 8, 16, "g3")
        cc_warm("w2")
        sc3, bi3 = gn_prep(ps45, e16_sb, 8, st3.rearrange("g b o s -> g (b o) s"), 8,
                           V_BG3G, V_BG3B, V_BG3C)
        for b in range(B):
            for ot in range(4):
                i = b * 4 + ot
                nc.scalar.activation(out=y_sb[ot][:, b, :], in_=y_sb[ot][:, b, :],
                                     func=AF.Relu, bias=bi3[:, i:i + 1], scale=sc3[:, i:i + 1])
            pf = ps45.tile([128, OC], F32, name="pf", tag="pf", bufs=2)
            for ot in range(4):
                nc.tensor.matmul(pf, r2d_sb[:, ot, :], y_sb[ot][:, b, :],
                                 start=(ot == 0), stop=(ot == 3))
            rot_copy(fpre[:, b, :], pf)
            stats_from(ps45, fpre[:, b, OWN_LO:OWN_HI], vcol(V_R2DB), g4_sb, 32, st2[:, b, :])
            nc.vector.memset(f_sb[b], 0.0)

        # ---- collective 2 + GN2 prep/apply ----
        cc_average(st2, 32, 2 * B, "g2")
        cc_warm("w3")
        sc2, bi2 = gn_prep(ps45, e4_sb, 32, st2, 2, V_BG2G, V_BG2B, V_BG2C)
        for b in range(B):
            nc.scalar.activation(out=f_sb[b][:, 1:TL + 1, 1:W + 1], in_=fpre[:, b, :],
                                 func=AF.Relu, bias=bi2[:, b:b + 1], scale=sc2[:, b:b + 1])
            # conv zero-padding at the global t boundary: the halo columns are
            # real data on interior cores, zero on edge cores (edge in {0,1}).
            for side in range(2):
                scE = small.tile([128, 1], F32, name="scE", tag="scE", bufs=2)
                biE = small.tile([128, 1], F32, name="biE", tag="biE", bufs=2)
                nc.vector.tensor_mul(scE, sc2[:, b:b + 1], edge_sb[:, side:side + 1])
                nc.vector.tensor_mul(biE, bi2[:, b:b + 1], edge_sb[:, side:side + 1])
                lt = 0 if side == 0 else TL - 1
                nc.scalar.activation(out=f_sb[b][:, lt + 1:lt + 2, 1:W + 1],
                                     in_=fpre[:, b, lt * W:(lt + 1) * W],
                                     func=AF.Relu, bias=biE, scale=scE)

        # ---- stage 5: heads ----
        for b in range(B):
            for hd in range(2):
                w_sb = s1w_sb if hd == 0 else e1w_sb
                i = b * 2 + hd
                phd = ps45.tile([128, TC * W], F32, name="phd", tag="phd", bufs=2)
                for tap in range(9):
                    kt, kw = tap // 3, tap % 3
                    # out t' in [0,32) maps to f_sb col (t'+1+kt, kw) in the
                    # (TL+2, W+2) padded layout
                    nc.tensor.matmul(phd, w_sb[:, tap, :],
                                     f_sb[b][:, 1 + kt:1 + kt + TC, kw:kw + W],
                                     start=(tap == 0), stop=(tap == 8))
                rot_copy(hpre[i], phd)
                stats_from(ps45, hpre[i], vcol(V_S1B + hd), g4_sb, 32, sth[:, b, hd, :])

        # ---- collective 3 + head GN + final 1x1 + sigmoid ----
        cc_average(sth, 32, 2 * B * 2, "gh")
        sch, bih = gn_prep(ps45, e4_sb, 32, sth.rearrange("g b h s -> g (b h) s"), 4,
                           V_BHG, V_BHB, V_BHC)
        for b in range(B):
            for hd in range(2):
                i = b * 2 + hd
                nc.scalar.activation(out=hpre[i], in_=hpre[i], func=AF.Relu,
                                     bias=bih[:, i:i + 1], scale=sch[:, i:i + 1])
                po = ps45.tile([1, TC * W], F32, name="po", tag="po", bufs=2)
                nc.tensor.matmul(po, s2w_sb[:, hd:hd + 1], hpre[i], start=True, stop=True)
                o_one = bigres.tile([1, TC * W], F32, name="o_one", tag="o_one", bufs=2)
                nc.scalar.activation(out=o_one, in_=po, func=AF.Sigmoid,
                                     bias=vec_sb[0:1, V_S2B + hd:V_S2B + hd + 1], scale=1.0)
                nc.sync.dma_start(out=outd[b, hd, :, :], in_=o_one)


# stage-1 batched GN prep needs bg1 vec rows; extend the table
V_BG1G = NVEC
V_BG1B = NVEC + 4
V_BG1C = NVEC + 8
NVEC_TOTAL = NVEC + 12


def sc_bias_col(mat, i):
    return mat[:, i:i + 1]


def _f32(a):
    return np.ascontiguousarray(np.asarray(a, dtype=np.float32))


def _prep_consts(inputs):
    c1_w = _f32(inputs["c1_w"])
    r3d_w = _f32(inputs["r3d_w"])[:, :, :, 0, 0]
    r2d_w = _f32(inputs["r2d_w"])[:, :, 0, 0]
    s1_w = _f32(inputs["s1_w"])
    e1_w = _f32(inputs["e1_w"])
    s2_w = _f32(inputs["s2_w"])[0, :, 0, 0]
    e2_w = _f32(inputs["e2_w"])[0, :, 0, 0]

    # conv1 weights: [c, j*4+ct, m] = c1_w[m, ct*128+c, j]
    a = c1_w.transpose(1, 2, 0).reshape(4, 128, 3, H1)
    c1w_h = np.ascontiguousarray(a.transpose(1, 2, 0, 3).reshape(128, 12, H1)).astype(BFNP)

    # r3d weights: [c, n*2+ct, o] = r3d_w[o, ct*128+c, n]
    a = r3d_w.transpose(1, 2, 0).reshape(2, 128, N, H3)
    r3d_h = np.ascontiguousarray(a.transpose(1, 2, 0, 3).reshape(128, 64, H3)).astype(BFNP)

    wtail = np.zeros((128, 23, H2), np.float32)
    wtail[:, WT_R2D:WT_R2D + 4, :] = r2d_w.T.reshape(4, 128, H2).transpose(1, 0, 2)
    wtail[:, WT_S1:WT_S1 + 9, :] = s1_w.transpose(1, 2, 3, 0).reshape(128, 9, H2)
    wtail[:, WT_E1:WT_E1 + 9, :] = e1_w.transpose(1, 2, 3, 0).reshape(128, 9, H2)
    wtail[:, WT_S2, 0] = s2_w
    wtail[:, WT_S2, 1] = e2_w
    wtail_h = wtail.astype(BFNP)

    ch = np.arange(128)
    g8 = (ch[:, None] // 8 == np.arange(16)[None, :]).astype(np.float32)
    g16 = (ch[:, None] // 16 == np.arange(8)[None, :]).astype(np.float32)
    g4 = (ch[:, None] // 4 == np.arange(32)[None, :]).astype(np.float32)
    gmats = np.concatenate([g8 / 8.0, g16 / 16.0, g4 / 4.0], axis=1)
    emats = np.zeros((96, 128), np.float32)
    emats[0:16] = g8.T
    emats[32:40] = g16.T
    emats[64:96] = g4.T

    vecs = np.zeros((NVEC_TOTAL, 128), np.float32)
    vecs[V_C1B:V_C1B + 2] = _f32(inputs["c1_b"]).reshape(2, 128)
    vecs[V_GN1G:V_GN1G + 2] = _f32(inputs["gn1_g"]).reshape(2, 128)
    vecs[V_GN1B:V_GN1B + 2] = _f32(inputs["gn1_b"]).reshape(2, 128)
    vecs[V_R3DB:V_R3DB + 4] = _f32(inputs["r3d_b"]).reshape(4, 128)
    vecs[V_GN3G:V_GN3G + 4] = _f32(inputs["gn3_g"]).reshape(4, 128)
    vecs[V_GN3B:V_GN3B + 4] = _f32(inputs["gn3_b"]).reshape(4, 128)
    vecs[V_R2DB] = _f32(inputs["r2d_b"])
    vecs[V_GN2G] = _f32(inputs["gn2_g"])
    vecs[V_GN2B] = _f32(inputs["gn2_b"])
    vecs[V_S1B] = _f32(inputs["s1_b"])
    vecs[V_E1B] = _f32(inputs["e1_b"])
    vecs[V_SGNG] = _f32(inputs["sgn_g"])
    vecs[V_SGNB] = _f32(inputs["sgn_b"])
    vecs[V_EGNG] = _f32(inputs["egn_g"])
    vecs[V_EGNB] = _f32(inputs["egn_b"])
    vecs[V_S2B] = _f32(inputs["s2_b"])[0]
    vecs[V_E2B] = _f32(inputs["e2_b"])[0]
    gn3g4 = _f32(inputs["gn3_g"]).reshape(4, 128)
    gn3b4 = _f32(inputs["gn3_b"]).reshape(4, 128)
    r3db4 = _f32(inputs["r3d_b"]).reshape(4, 128)
    for i, (b, ot) in enumerate([(b, ot) for b in range(B) for ot in range(4)]):
        vecs[V_BG3G + i] = gn3g4[ot]
        vecs[V_BG3B + i] = gn3b4[ot]
        vecs[V_BG3C + i] = r3db4[ot]
    for b in range(B):
        vecs[V_BG2G + b] = _f32(inputs["gn2_g"])
        vecs[V_BG2B + b] = _f32(inputs["gn2_b"])
        vecs[V_BG2C + b] = _f32(inputs["r2d_b"])
    hg = [_f32(inputs["sgn_g"]), _f32(inputs["egn_g"])]
    hb = [_f32(inputs["sgn_b"]), _f32(inputs["egn_b"])]
    hc = [_f32(inputs["s1_b"]), _f32(inputs["e1_b"])]
    for i, (b, hd) in enumerate([(b, hd) for b in range(B) for hd in range(2)]):
        vecs[V_BHG + i] = hg[hd]
        vecs[V_BHB + i] = hb[hd]
        vecs[V_BHC + i] = hc[hd]
    gn1g2 = _f32(inputs["gn1_g"]).reshape(2, 128)
    gn1b2 = _f32(inputs["gn1_b"]).reshape(2, 128)
    c1b2 = _f32(inputs["c1_b"]).reshape(2, 128)
    for i, (b, mt) in enumerate([(b, mt) for b in range(B) for mt in range(2)]):
        vecs[V_BG1G + i] = gn1g2[mt]
        vecs[V_BG1B + i] = gn1b2[mt]
        vecs[V_BG1C + i] = c1b2[mt]

    return {
        "c1w": c1w_h, "r3dw": r3d_h, "wtail": wtail_h,
        "gmats": gmats, "emats": emats, "vecs": vecs,
        "mask": _f32(inputs["sample_mask"]).reshape(T, N, T, W),
    }


def _fingerprint(inputs):
    h = hashlib.sha1()
    for k in sorted(inputs.keys()):
        if k == "x":
            continue
        a = np.asarray(inputs[k])
        h.update(k.encode())
        h.update(str(a.shape).encode())
        h.update(str(a.dtype).encode())
        flat = a.reshape(-1)
        step = max(1, flat.size // 65536)
        h.update(np.ascontiguousarray(flat[::step]).tobytes())
    return h.hexdigest()


_module_cache = {}


def _get_module(inputs=None):
    if inputs is None:
        if "nc" not in _module_cache:
            raise RuntimeError("module not built yet; call kernel() first")
        return _module_cache["nc"]
    fp = _fingerprint(inputs)
    if _module_cache.get("fp") != fp:
        consts = _prep_consts(inputs)
        _module_cache["nc"] = _build(consts)
        _module_cache["fp"] = fp
        _module_cache["consts"] = consts
    return _module_cache["nc"]


def _prep(inputs):
    """Per-core input maps: full x, per-core mask slice + edge mask."""
    x_h = np.ascontiguousarray(_f32(inputs["x"]).astype(BFNP))
    mask = _f32(inputs["sample_mask"]).reshape(T, N, T, W)
    maps = []
    for c in range(NCORES):
        mp = np.zeros((2, 128, N, OC), np.float32)
        for lt in range(TL):
            gt = TC * c - 1 + lt
            if 0 <= gt < T:
                # mp[tt, p, n, lt*W+w] = mask[tt*128+p, n, gt, w]
                mslice = mask[:, :, gt, :]  # (T', N, W)
                mp[:, :, :, lt * W:(lt + 1) * W] = mslice.reshape(2, 128, N, W)
        edge = np.ones((128, 2), np.float32)
        if c == 0:
            edge[:, 0] = 0.0
        if c == NCORES - 1:
            edge[:, 1] = 0.0
        maps.append({
            "x_in": x_h,
            "maskp": np.ascontiguousarray(mp.astype(BFNP)),
            "edge": edge,
        })
    return maps


def kernel(**inputs) -> np.ndarray:
    nc = _get_module(inputs)
    in_maps = _prep(inputs)
    from concourse.bass_utils import run_bass_kernel_spmd
    res = run_bass_kernel_spmd(nc, in_maps, list(range(NCORES)))
    full = np.zeros((B, 2, T, W), np.float32)
    for c in range(NCORES):
        full[:, :, TC * c:TC * (c + 1), :] = res.results[c]["out"].astype(np.float32)
    return np.ascontiguousarray(full)
